# revision 1
# baseline (speedup 1.0000x reference)
"""Trainium2 Bass kernel v2 for fused ragged attention pooling.

Same math as the baseline (see kernel.py docstring) with a restructured
device schedule:

- scores matmuls col-tiled 4-way: 4 pairs of graphs run concurrently in
  four 32-column strips of the PE array (output is only 8 rows per pair).
- pooling matmuls col-tiled 4-way: block-diag p covers 4 graphs (32 rows)
  per strip; 4 graph-sets stream their x chunks concurrently.
- quad-wide softmax: scores padded to 32 rows per strip so exp (ACT) and
  the 1/denominator scale (DVE) run on [128, 256] tiles (2 ops per quad).
- PSUM->SBUF evacuation of transposed x in full-bank tiles, rotated 2:1
  across DVE and ACT (GPSIMD cannot reach PSUM).
- x pre-tiled on host so each per-set DMA (1 MB, 4 graphs) is a fully
  contiguous 8KB-per-partition read.

Per core: 64 graphs = 4 supergroups x 16 graphs; supergroup = 2 quads of
4 pairs; set = 4 graphs (2 pairs) sharing one pooling strip.
"""

import numpy as np

N, D, B, H = 131072, 512, 512, 8
DH = D // H
CORES = 8
GPC = B // CORES          # graphs per core = 64
NPG = N // B              # nodes per graph = 256
SG = 16                   # graphs per supergroup
NSETS = 4                 # sets per supergroup (4 graphs each)

_CACHE = {}

CONF = {
    "xbufs": 8,        # xset tiles [128, 8, 512] f16 in flight
    "xtsbbufs": 6,     # per-pair x^T staging tiles
    "xtpsbufs": 3,     # PSUM transpose tiles (1 bank each)
    "scbufs": 2,       # PSUM score quad tiles (1 bank each)
    "ppgbufs": 1,      # PSUM p-natural tiles (1 bank each)
    "s2bufs": 1,       # PSUM pooled tiles (1 bank each)
    "p16bufs": 8,      # persistent p block-diag tiles (2 supergroups x 4 sets)
    "evac": "vva",     # rotation of evacuation engines: v=DVE a=ACT (GPSIMD cannot reach PSUM)
    "xchunk": 1,       # sets per x DMA (1 = 1MB loads, 2 = 2MB, 4 = 4MB)
    "bigxtps": 1,      # 1 = full-bank transpose tiles, 1 evac copy per half-pair
    "dmasplit": 0,     # alternate x DMAs across SP/ACT HWDGE rings (sim: ACT-ring FIFO delays exp)
    "interleave": 1,   # interleave pooling/scores MMs across strips
}


def _in_maps(x, A4, WvT4, Wout8, idrep8, conf=None):
    x = x.astype(np.float16)
    ident = np.eye(128, dtype=np.float32)
    npc = GPC * NPG
    nsets = GPC // 4
    # pre-tile to [set, partition, chunk, d] so device DMAs are contiguous
    xt = np.ascontiguousarray(
        x.reshape(CORES, nsets, 8, 128, D).transpose(0, 1, 3, 2, 4)
    )
    return [
        {
            "x": xt[c],
            "a4": A4.astype(np.float16),
            "wvt4": WvT4.astype(np.float16),
            "wout8": Wout8,
            "identr": ident.astype(np.float16),
            "idrep8": idrep8.astype(np.float16),
        }
        for c in range(CORES)
    ]


def _build(n_graphs, repeat=1, variant="full", **overrides):
    conf = dict(CONF, **overrides)
    from contextlib import ExitStack, nullcontext

    import concourse.bacc as bacc
    import concourse.tile as tile
    from concourse import mybir

    F16 = mybir.dt.float16
    F32 = mybir.dt.float32
    F32R = mybir.dt.float32r
    U32 = mybir.dt.uint32
    EXP = mybir.ActivationFunctionType.Exp

    assert n_graphs % SG == 0
    n_sg = n_graphs // SG

    nc = bacc.Bacc("TRN2", target_bir_lowering=False, debug=False)

    x_d = nc.dram_tensor(
        "x", [n_graphs // 4, 128, 8, D], F16, kind="ExternalInput"
    )
    a_d = nc.dram_tensor("a4", [128, 4, 32], F16, kind="ExternalInput")
    wv_d = nc.dram_tensor("wvt4", [128, 4, H, DH], F16, kind="ExternalInput")
    wo_d = nc.dram_tensor("wout8", [DH, H, D], F32R, kind="ExternalInput")
    idr_d = nc.dram_tensor("identr", [128, 128], F16, kind="ExternalInput")
    idrep_d = nc.dram_tensor("idrep8", [128, H], F16, kind="ExternalInput")
    out_d = nc.dram_tensor("out", [n_graphs, D], F32, kind="ExternalOutput")

    evac_engines = {
        "v": lambda o, i: nc.vector.tensor_copy(o, i),
        "a": lambda o, i: nc.scalar.copy(o, i),
        "p": lambda o, i: nc.gpsimd.tensor_copy(o, i),
    }
    evac_cycle = [evac_engines[ch] for ch in conf["evac"]]

    with tile.TileContext(nc) as tc, ExitStack() as ctx:
        const = ctx.enter_context(tc.tile_pool(name="const", bufs=1))
        xpool = ctx.enter_context(tc.tile_pool(name="x", bufs=conf["xbufs"]))
        xtsb_pool = ctx.enter_context(tc.tile_pool(name="xtsb", bufs=conf["xtsbbufs"]))
        eq_pool = ctx.enter_context(
            tc.tile_pool(name="eq", bufs=conf.get("eqbufs", 2))
        )
        pt_pool = ctx.enter_context(
            tc.tile_pool(name="pt", bufs=conf.get("eqbufs", 2))
        )
        den_pool = ctx.enter_context(tc.tile_pool(name="den", bufs=4))
        p16_pool = ctx.enter_context(tc.tile_pool(name="p16", bufs=1))
        s2sb_pool = ctx.enter_context(tc.tile_pool(name="s2sb", bufs=2))
        stall_pool = ctx.enter_context(tc.tile_pool(name="stall", bufs=1))
        tail_sb = ctx.enter_context(tc.tile_pool(name="tailsb", bufs=1))

        xtps_pool = ctx.enter_context(
            tc.tile_pool(name="xtps", bufs=conf["xtpsbufs"], space="PSUM")
        )
        scps_pool = ctx.enter_context(
            tc.tile_pool(name="scps", bufs=conf["scbufs"], space="PSUM")
        )
        ppg_pool = ctx.enter_context(
            tc.tile_pool(name="ppg", bufs=conf["ppgbufs"], space="PSUM")
        )
        s2ps_pool = ctx.enter_context(
            tc.tile_pool(name="s2ps", bufs=conf["s2bufs"], space="PSUM")
        )
        tail_ps = ctx.enter_context(tc.tile_pool(name="tailps", bufs=1, space="PSUM"))

        # A padded to 32 output rows per strip (cols 8..31 zero) so the
        # score strips fill all 32 partitions -> quad-wide exp/scale ops.
        A4 = const.tile([128, 4, 32], F16)
        nc.scalar.dma_start(A4[:], a_d[:])
        WvT4 = const.tile([128, 4, H, DH], F16)
        nc.scalar.dma_start(WvT4[:], wv_d[:])
        Wout8 = const.tile([DH, H, D], F32R)
        nc.scalar.dma_start(Wout8[:], wo_d[:])
        identr = const.tile([128, 128], F16)
        nc.scalar.dma_start(identr[:], idr_d[:])
        # idrep8[32c:32c+8, :] = I8 for each strip c (transpose rhs must be
        # a square permutation matrix).
        idrep8 = const.tile([128, H], F16)
        nc.scalar.dma_start(idrep8[:], idrep_d[:])

        if variant != "dma":
            STall = stall_pool.tile([128, 4, n_sg, 128], F16)
            # persistent block-diag p tiles: off-diagonal zeros are written
            # once here; the scatter rewrites only the diagonal slots, so
            # zeros stay valid across supergroups and For_i iterations.
            P16all = [
                p16_pool.tile([128, 8, 32], F16, tag=f"p16_{i}", name=f"p16_{i}")
                for i in range(min(2, n_sg) * NSETS)
            ]
            for t in P16all:
                nc.vector.memset(t[:].bitcast(U32), 0)

        evac_idx = [0]

        def evac(dst, src):
            evac_cycle[evac_idx[0] % len(evac_cycle)](dst, src)
            evac_idx[0] += 1

        loop_cm = tc.For_i(0, repeat, 1) if repeat > 1 else nullcontext()
        with loop_cm:
            for sg in range(n_sg):
                # ---- load 4 sets (4 graphs each), xchunk sets per DMA ----
                xc = conf["xchunk"]
                xs = []
                for cp0 in range(0, NSETS, xc):
                    xt = xpool.tile([128, xc, 8, D], F16, tag="x", name=f"xs{cp0}")
                    if variant != "nodma":
                        s0 = sg * NSETS + cp0
                        eng = (
                            nc.scalar
                            if conf["dmasplit"] and (cp0 // xc) % 2 == 1
                            else nc.sync
                        )
                        eng.dma_start(
                            xt[:],
                            x_d[s0 : s0 + xc, :, :, :].rearrange(
                                "s p a d -> p s a d"
                            ),
                        )
                    for i in range(xc):
                        xs.append(xt[:, i, :, :])
                if variant == "dma":
                    continue

                p16 = P16all[
                    (sg % 2) * NSETS : (sg % 2) * NSETS + NSETS
                ] if n_sg > 1 else P16all

                for q in range(2):
                    # ---- transposes for the quad's 4 pairs ----
                    xtsbs = []
                    for cs in range(4):
                        p = 4 * q + cs
                        cset, j = p // 2, p % 2
                        xsrc = xs[cset]
                        xtsb = xtsb_pool.tile([128, 4, 4, 128], F16, tag="xtsb")
                        if conf["bigxtps"]:
                            for kk in range(2):
                                xtA = xtps_pool.tile(
                                    [128, 4, 2, 128], F16, tag="xt"
                                )
                                for k in range(2):
                                    for c in range(4):
                                        nc.tensor.matmul(
                                            xtA[:, c, k, :],
                                            xsrc[
                                                :, 4 * j + 2 * kk + k,
                                                128 * c : 128 * (c + 1),
                                            ],
                                            identr[:],
                                            is_transpose=True,
                                        )
                                evac(xtsb[:, :, 2 * kk : 2 * kk + 2, :], xtA[:])
                        else:
                            for kk in range(2):
                                xtA = xtps_pool.tile([128, 2, 2, 128], F16, tag="xt")
                                xtB = xtps_pool.tile([128, 2, 2, 128], F16, tag="xt")
                                for k in range(2):
                                    for c in range(4):
                                        dst = xtA if c < 2 else xtB
                                        nc.tensor.matmul(
                                            dst[:, c % 2, k, :],
                                            xsrc[
                                                :, 4 * j + 2 * kk + k,
                                                128 * c : 128 * (c + 1),
                                            ],
                                            identr[:],
                                            is_transpose=True,
                                        )
                                evac(xtsb[:, 0:2, 2 * kk : 2 * kk + 2, :], xtA[:])
                                evac(xtsb[:, 2:4, 2 * kk : 2 * kk + 2, :], xtB[:])
                        xtsbs.append(xtsb)

                    # ---- scores: 16 MMs interleaved across 4 strips ----
                    scps = scps_pool.tile([128, 512], F32, tag="scps")
                    for ci in range(4):
                        for cs in range(4):
                            nc.tensor.matmul(
                                scps[32 * cs : 32 * cs + 32, :],
                                A4[:, ci, :],
                                xtsbs[cs][:, ci, :, :],
                                start=(ci == 0),
                                stop=(ci == 3),
                                skip_group_check=True,
                                tile_position=(0, 32 * cs),
                            )

                    # ---- softmax, quad-wide (all 4 strips at once) ----
                    denq = den_pool.tile([128, 2], F32, tag="den")
                    eq = eq_pool.tile([128, 2, NPG], F16, tag="eq")
                    for g in range(2):
                        nc.scalar.activation(
                            eq[:, g, :],
                            scps[:, NPG * g : NPG * (g + 1)],
                            EXP,
                            accum_out=denq[:, g : g + 1],
                        )
                    rdenq = den_pool.tile([128, 2], F32, tag="rden")
                    nc.vector.reciprocal(rdenq[:], denq[:])
                    ptq = pt_pool.tile([128, 2, NPG], F16, tag="ptq")
                    for g in range(2):
                        nc.vector.tensor_scalar_mul(
                            ptq[:, g, :],
                            eq[:, g, :],
                            rdenq[:, g : g + 1],
                        )

                    # ---- p back to natural layout + block-diag scatter ----
                    # (strip-sequential: interleaving transposes across row
                    # groups hangs on HW even though it compiles + sims)
                    from concourse.ap import AP as _AP

                    for cs in range(4):
                        p = 4 * q + cs
                        cset, j = p // 2, p % 2
                        ppg = ppg_pool.tile([128, 2, 2, H], F16, tag="ppg")
                        for g in range(2):
                            for k in range(2):
                                nc.tensor.matmul(
                                    ppg[:, g, k, :],
                                    ptq[
                                        32 * cs : 32 * cs + 8, g,
                                        128 * k : 128 * (k + 1),
                                    ],
                                    idrep8[32 * cs : 32 * cs + 8, :],
                                    is_transpose=True,
                                    tile_position=(32 * cs, 0),
                                )
                        # p16 free idx = kset*32 + 4h + gl, kset = 4j+2g+k,
                        # gl = 2j+g  ->  g*65 + k*32 + h*4 + 130*j
                        base = p16[cset][:]
                        dst = _AP(
                            base.tensor,
                            base.offset + 130 * j,
                            [list(base.ap[0])] + [[65, 2], [32, 2], [4, H]],
                        )
                        nc.vector.tensor_copy(dst, ppg[:])

                # ---- pooling: 32 MMs interleaved across 4 strips ----
                s2ps = s2ps_pool.tile([128, D], F32, tag="s2")
                for k in range(8):
                    for cp in range(NSETS):
                        nc.tensor.matmul(
                            s2ps[32 * cp : 32 * cp + 32, :],
                            p16[cp][:, k, :],
                            xs[cp][:, k, :],
                            start=(k == 0),
                            stop=(k == 7),
                            skip_group_check=True,
                            tile_position=(0, 32 * cp),
                        )

                # ---- evacuate pooled S, transpose for the tail ----
                s2sb = s2sb_pool.tile([128, D], F16, tag="s2sb")
                nc.vector.tensor_copy(s2sb[:], s2ps[:])
                stps = tail_ps.tile([128, 4, 128], F16, tag="tail")
                for c in range(4):
                    nc.tensor.matmul(
                        stps[:, c, :],
                        s2sb[:, 128 * c : 128 * (c + 1)],
                        identr[:],
                        is_transpose=True,
                    )
                nc.vector.tensor_copy(STall[:, :, sg, :], stps[:])

            if variant == "dma":
                finz = tail_sb.tile([n_graphs, D], F32, tag="finsb")
                nc.vector.memset(finz[:], 0.0)
                nc.sync.dma_start(out_d[:], finz[:])
            else:
                # pooledT[j, graphs] per head; STall col idx = 32*set+4h+gl
                pool4 = tail_ps.tile([DH, H, n_graphs], F32, tag="tail")
                for h in range(H):
                    for c in range(4):
                        rhs = STall[:, c, :, :].rearrange(
                            "p s (cp h gl) -> p s cp h gl", cp=4, h=H, gl=4
                        )[:, :, :, h, :]
                        nc.tensor.matmul(
                            pool4[:, h, :],
                            WvT4[:, c, h, :],
                            rhs,
                            start=(c == 0),
                            stop=(c == 3),
                        )
                pool4sb = tail_sb.tile([DH, H, n_graphs], F32R, tag="p4sb")
                nc.vector.tensor_copy(pool4sb[:], pool4[:])
                finps = tail_ps.tile([n_graphs, D], F32, tag="tail")
                for h in range(H):
                    nc.tensor.matmul(
                        finps[:],
                        pool4sb[:, h, :],
                        Wout8[:, h, :],
                        start=(h == 0),
                        stop=(h == H - 1),
                    )
                finsb = tail_sb.tile([n_graphs, D], F32, tag="finsb")
                nc.vector.tensor_copy(finsb[:], finps[:])
                nc.sync.dma_start(out_d[:], finsb[:])

    nc.compile()
    _strip_debug(nc)
    return nc


def _strip_debug(nc):
    for fn in nc.m.functions:
        for alloc in fn.allocations:
            try:
                for ml in alloc.memorylocations or []:
                    if getattr(ml, "ant_debug", None) is not None:
                        ml.ant_debug = None
            except Exception:
                pass
        for b in fn.blocks:
            for inst in b.instructions:
                try:
                    if inst.debug is not None:
                        inst.debug = None
                    if inst.bass_addl_debug is not None:
                        inst.bass_addl_debug = None
                except Exception:
                    pass


def _host_prep(query, W_in, b_in, W_out, b_out):
    scale = 1.0 / np.sqrt(DH)
    q = ((query @ W_in[:D].T + b_in[:D]) * scale).reshape(H, DH)
    Wk = W_in[D : 2 * D]
    A = (Wk.reshape(H, DH, D) * q[:, :, None]).sum(1).T.astype(np.float32)
    A4 = np.zeros((128, 4, 32), np.float32)
    A4[:, :, :H] = A.reshape(4, 128, H).transpose(1, 0, 2)
    WvT = W_in[2 * D :].T.astype(np.float32)
    WvT4 = np.ascontiguousarray(WvT.reshape(4, 128, H, DH).transpose(1, 0, 2, 3))
    WoutT = W_out.T.astype(np.float32)
    Wout8 = np.ascontiguousarray(WoutT.reshape(H, DH, D).transpose(1, 0, 2))
    bias = (W_out @ b_in[2 * D :] + b_out).astype(np.float32)
    idrep8 = np.zeros((128, H), np.float32)
    for c in range(4):
        idrep8[32 * c : 32 * c + H, :] = np.eye(H)
    return A4, WvT4, Wout8, bias, idrep8


def _numpy_fallback(x, batch, num_graphs, query, W_in, b_in, W_out, b_out):
    nb = int(num_graphs)
    scale = 1.0 / np.sqrt(DH)
    q = ((query @ W_in[:D].T + b_in[:D]) * scale).reshape(H, DH)
    k = (x @ W_in[D : 2 * D].T + b_in[D : 2 * D]).reshape(-1, H, DH)
    v = (x @ W_in[2 * D :].T + b_in[2 * D :]).reshape(-1, H, DH)
    scores = np.einsum("nhd,hd->nh", k, q)
    smax = np.full((nb, H), -np.inf, np.float32)
    np.maximum.at(smax, batch, scores)
    e = np.exp(scores - smax[batch])
    denom = np.zeros((nb, H), np.float32)
    np.add.at(denom, batch, e)
    p = e / denom[batch]
    pooled = np.zeros((nb, H, DH), np.float32)
    np.add.at(pooled, batch, p[:, :, None] * v)
    return (pooled.reshape(nb, D) @ W_out.T + b_out).astype(np.float32)


def kernel(**inputs):
    x = np.ascontiguousarray(np.asarray(inputs["x"], dtype=np.float32))
    batch = np.asarray(inputs["batch"]).astype(np.int64)
    num_graphs = int(np.asarray(inputs["num_graphs"]))
    query = np.asarray(inputs["query"], dtype=np.float32)
    W_in = np.asarray(inputs["W_in"], dtype=np.float32)
    b_in = np.asarray(inputs["b_in"], dtype=np.float32)
    W_out = np.asarray(inputs["W_out"], dtype=np.float32)
    b_out = np.asarray(inputs["b_out"], dtype=np.float32)

    regular = (
        x.shape == (N, D)
        and num_graphs == B
        and batch.shape == (N,)
        and np.array_equal(batch, np.repeat(np.arange(B, dtype=np.int64), NPG))
    )
    if not regular:
        return _numpy_fallback(
            x, batch, num_graphs, query, W_in, b_in, W_out, b_out
        )

    from concourse.bass_utils import run_bass_kernel_spmd

    A4, WvT4, Wout8, bias, idrep8 = _host_prep(query, W_in, b_in, W_out, b_out)

    if "prog" not in _CACHE:
        _CACHE["prog"] = _build(GPC)
    nc = _CACHE["prog"]

    in_maps = _in_maps(x, A4, WvT4, Wout8, idrep8)
    res = run_bass_kernel_spmd(nc, in_maps, list(range(CORES)))
    out = np.concatenate([res.results[c]["out"] for c in range(CORES)], axis=0)
    return (out + bias[None, :]).astype(np.float32)



# revision 2
# speedup vs baseline: 16.5403x; 16.5403x over previous
"""Trainium2 Bass kernel v3 for fused ragged attention pooling.

Same math as v2 with a software-pipelined device schedule:

- stage skew across quads: transposes T(q) | scores+softmax S,X(q-1) |
  p-transpose+scatter P(q-2) and pooling L(sg) - so the PE never waits
  on the ACT/DVE softmax chain mid-quad.
- scores matmuls use 8-column stationaries (s8): less LDWEIGHTS work.
- the out DMA is deferred by one repeat-loop iteration so the in-order
  sync DMA ring never stalls next-iteration x prefetch on the tail
  (flushed after the loop; single-shot builds write directly).
- scores matmuls col-tiled 4-way via tile_position (4 strips of the PE
  array run concurrently); pooling likewise with block-diag p.
- quad-wide softmax on [128, 256] tiles; PSUM evacuation rotated 2:1
  across DVE and ACT (only those engines reach PSUM).
- x pre-tiled on host so each per-set DMA (1 MB) is a contiguous
  8KB-per-partition read.

Per core: 64 graphs = 4 supergroups x 16 graphs; supergroup = 2 quads of
4 pairs; set = 4 graphs (2 pairs) sharing one pooling strip.
"""

import numpy as np

N, D, B, H = 131072, 512, 512, 8
DH = D // H
CORES = 8
GPC = B // CORES          # graphs per core = 64
NPG = N // B              # nodes per graph = 256
SG = 16                   # graphs per supergroup
NSETS = 4                 # sets per supergroup (4 graphs each)

_CACHE = {}

CONF = {
    "xbufs": 12,       # xset tiles [128, 8, 512] f16 in flight (3 sg)
    "xtsbbufs": 9,     # per-pair x^T staging tiles (2 quads + 1)
    "xtpsbufs": 3,     # PSUM transpose tiles (1 bank each)
    "scbufs": 2,       # PSUM score quad tiles (1 bank each)
    "ppgbufs": 1,      # PSUM p-natural tiles (1 bank each)
    "s2bufs": 1,       # PSUM pooled tiles (1 bank each)
    "eqbufs": 3,
    "ptbufs": 3,
    "skew": 2,         # stage skew in quads: T(q) | S,X(q-1) | P(q-2),L
    "evac": "vva",     # rotation of evacuation engines: v=DVE a=ACT
    "xchunk": 1,       # sets per x DMA (1 = 1MB loads, 2 = 2MB, 4 = 4MB)
    "xtq": 0,          # quads (of n_q) whose x^T loads pre-transposed from DRAM
    "s8": 1,           # scores stationaries use 8 real head columns only
    "deferout": 1,     # out DMA deferred one loop iteration (ring-stall fix)
}


def _in_maps(x, A4, WvT4, Wout8, idrep8, conf=None):
    conf = dict(CONF, **(conf or {}))
    x = x.astype(np.float16)
    ident = np.eye(128, dtype=np.float32)
    nsets = GPC // 4
    # pre-tile to [set, partition, chunk, d] so device DMAs are contiguous
    xt = np.ascontiguousarray(
        x.reshape(CORES, nsets, 8, 128, D).transpose(0, 1, 3, 2, 4)
    )
    maps = [
        {
            "x": xt[c],
            "a4": A4.astype(np.float16),
            "wvt4": WvT4.astype(np.float16),
            "wout8": Wout8,
            "identr": ident.astype(np.float16),
            "idrep8": idrep8.astype(np.float16),
        }
        for c in range(CORES)
    ]
    xtq = conf["xtq"]
    if xtq:
        n_q = 2 * (GPC // SG)
        pre_quads = {round(i * n_q / xtq + (n_q / xtq - 1) / 2) for i in range(xtq)}
        pre_quads = sorted(pre_quads)[:xtq]
        # xtsb[dp, ci, k, n] = x_pair[128k + n, 128ci + dp] per pair
        xp = x.reshape(CORES, GPC // 2, 4, 128, 4, 128)  # [c, pair, k, n, ci, dp]
        for c in range(CORES):
            pres = []
            for q in pre_quads:
                for cs in range(4):
                    pair = 4 * q + cs
                    pres.append(
                        xp[c, pair].transpose(3, 2, 0, 1)  # [dp, ci, k, n]
                    )
            maps[c]["xtpre"] = np.ascontiguousarray(np.stack(pres))
    return maps


def _build(n_graphs, repeat=1, variant="full", **overrides):
    conf = dict(CONF, **overrides)
    from contextlib import ExitStack, nullcontext

    import concourse.bacc as bacc
    import concourse.tile as tile
    from concourse import mybir

    F16 = mybir.dt.float16
    F32 = mybir.dt.float32
    F32R = mybir.dt.float32r
    U32 = mybir.dt.uint32
    EXP = mybir.ActivationFunctionType.Exp

    assert n_graphs % SG == 0
    n_sg = n_graphs // SG
    n_q = 2 * n_sg  # quads per iteration

    nc = bacc.Bacc("TRN2", target_bir_lowering=False, debug=False)

    x_d = nc.dram_tensor(
        "x", [n_graphs // 4, 128, 8, D], F16, kind="ExternalInput"
    )
    a_d = nc.dram_tensor("a4", [128, 4, 32], F16, kind="ExternalInput")
    wv_d = nc.dram_tensor("wvt4", [128, 4, H, DH], F16, kind="ExternalInput")
    wo_d = nc.dram_tensor("wout8", [DH, H, D], F32R, kind="ExternalInput")
    idr_d = nc.dram_tensor("identr", [128, 128], F16, kind="ExternalInput")
    idrep_d = nc.dram_tensor("idrep8", [128, H], F16, kind="ExternalInput")
    out_d = nc.dram_tensor("out", [n_graphs, D], F32, kind="ExternalOutput")
    xtq = conf["xtq"]
    pre_quads = set()
    if xtq:
        # spread pre-transposed quads across the iteration
        pre_quads = {round(i * n_q / xtq + (n_q / xtq - 1) / 2) for i in range(xtq)}
        pre_quads = set(list(pre_quads)[:xtq])
        xt_d = nc.dram_tensor(
            "xtpre", [len(pre_quads) * 4, 128, 4, 4, 128], F16,
            kind="ExternalInput",
        )
        pre_index = {q: i for i, q in enumerate(sorted(pre_quads))}

    evac_engines = {
        "v": lambda o, i: nc.vector.tensor_copy(o, i),
        "a": lambda o, i: nc.scalar.copy(o, i),
    }
    evac_cycle = [evac_engines[ch] for ch in conf["evac"]]

    with tile.TileContext(nc) as tc, ExitStack() as ctx:
        const = ctx.enter_context(tc.tile_pool(name="const", bufs=1))
        xpool = ctx.enter_context(tc.tile_pool(name="x", bufs=conf["xbufs"]))
        xtsb_pool = ctx.enter_context(
            tc.tile_pool(name="xtsb", bufs=conf["xtsbbufs"])
        )
        eq_pool = ctx.enter_context(
            tc.tile_pool(name="eq", bufs=conf.get("eqbufs", 3))
        )
        pt_pool = ctx.enter_context(
            tc.tile_pool(name="pt", bufs=conf.get("ptbufs", 3))
        )
        den_pool = ctx.enter_context(tc.tile_pool(name="den", bufs=6))
        p16_pool = ctx.enter_context(tc.tile_pool(name="p16", bufs=1))
        s2sb_pool = ctx.enter_context(tc.tile_pool(name="s2sb", bufs=2))
        stall_pool = ctx.enter_context(tc.tile_pool(name="stall", bufs=1))
        tail_sb = ctx.enter_context(tc.tile_pool(name="tailsb", bufs=2))

        xtps_pool = ctx.enter_context(
            tc.tile_pool(name="xtps", bufs=conf["xtpsbufs"], space="PSUM")
        )
        scps_pool = ctx.enter_context(
            tc.tile_pool(name="scps", bufs=conf["scbufs"], space="PSUM")
        )
        ppg_pool = ctx.enter_context(
            tc.tile_pool(name="ppg", bufs=conf["ppgbufs"], space="PSUM")
        )
        s2ps_pool = ctx.enter_context(
            tc.tile_pool(name="s2ps", bufs=conf["s2bufs"], space="PSUM")
        )
        tail_ps = ctx.enter_context(
            tc.tile_pool(name="tailps", bufs=1, space="PSUM")
        )

        A4 = const.tile([128, 4, 32], F16)
        nc.scalar.dma_start(A4[:], a_d[:])
        WvT4 = const.tile([128, 4, H, DH], F16)
        nc.scalar.dma_start(WvT4[:], wv_d[:])
        Wout8 = const.tile([DH, H, D], F32R)
        nc.scalar.dma_start(Wout8[:], wo_d[:])
        identr = const.tile([128, 128], F16)
        nc.scalar.dma_start(identr[:], idr_d[:])
        idrep8 = const.tile([128, H], F16)
        nc.scalar.dma_start(idrep8[:], idrep_d[:])

        if variant != "dma":
            STall = stall_pool.tile([128, 4, n_sg, 128], F16)
            P16all = [
                p16_pool.tile([128, 8, 32], F16, tag=f"p16_{i}", name=f"p16_{i}")
                for i in range(min(2, n_sg) * NSETS)
            ]
            for t in P16all:
                nc.vector.memset(t[:].bitcast(U32), 0)

        if variant == "nodma":
            xs_static = []
            for hh in range(2):
                group = []
                for s in range(NSETS):
                    t = xpool.tile(
                        [128, 1, 8, D], F16, tag="x", name=f"xst{hh}_{s}"
                    )
                    nc.vector.memset(t[:], 1.0)
                    group.append(t[:, 0, :, :])
                xs_static.append(group)

        ablate = set(conf.get("ablate", ""))
        if ablate and variant == "full":
            ab_pool = ctx.enter_context(tc.tile_pool(name="abps", bufs=1, space="PSUM"))
            ab_sb = ctx.enter_context(tc.tile_pool(name="absb", bufs=1))
            U32_ = U32
            if "T" in ablate:
                xtA_static = [
                    ab_pool.tile([128, 4, 2, 128], F16, tag=f"abxt{i}",
                                 name=f"abxt{i}")
                    for i in range(2)
                ]
                for t in xtA_static:
                    nc.vector.memset(t[:].bitcast(U32_), 0)
            if "E" in ablate:
                xtsb_static = ab_sb.tile(
                    [128, 4, 4, 128], F16, name="abxtsb"
                )
                nc.vector.memset(xtsb_static[:].bitcast(U32_), 0)
            if "S" in ablate:
                scps_static = ab_pool.tile([128, 512], F32, name="abscps")
                nc.vector.memset(scps_static[:], 0.5)
            if "X" in ablate:
                ptq_static = ab_sb.tile([128, 2, NPG], F16, name="abptq")
                nc.vector.memset(ptq_static[:].bitcast(U32_), 0)
            if "P" in ablate:
                ppg_static = ab_pool.tile([128, 2, 2, H], F16, name="abppg")
                nc.vector.memset(ppg_static[:].bitcast(U32_), 0)
            if "L" in ablate:
                s2ps_static = ab_pool.tile([128, D], F32, name="abs2")
                nc.vector.memset(s2ps_static[:], 0.5)
            if "O" in ablate:
                fin_static = ab_sb.tile([n_graphs, D], F32, name="abfin")
                nc.vector.memset(fin_static[:], 0.0)

        evac_idx = [0]

        def evac(dst, src):
            evac_cycle[evac_idx[0] % len(evac_cycle)](dst, src)
            evac_idx[0] += 1

        from concourse.ap import AP as _AP

        prev_finsb = []
        loop_cm = tc.For_i(0, repeat, 1) if repeat > 1 else nullcontext()
        with loop_cm:
            # per-quad pipeline state
            xs_all = {}     # sg -> list of 4 set-slices of x natural
            xtsbs_q = {}    # q -> list of 4 xtsb tiles
            scps_q = {}     # q -> scores PSUM tile
            ptq_q = {}      # q -> normalized p^T tile

            def emit_dma(sg):
                if variant == "nodma":
                    xs_all[sg] = xs_static[sg % 2]
                    return
                xc = conf["xchunk"]
                xs = []
                for cp0 in range(0, NSETS, xc):
                    xt = xpool.tile(
                        [128, xc, 8, D], F16, tag="x", name=f"xs{cp0}"
                    )
                    s0 = sg * NSETS + cp0
                    nc.sync.dma_start(
                        xt[:],
                        x_d[s0 : s0 + xc, :, :, :].rearrange(
                            "s p a d -> p s a d"
                        ),
                    )
                    for i in range(xc):
                        xs.append(xt[:, i, :, :])
                xs_all[sg] = xs

            def emit_T(q):
                # transposes for quad q's 4 pairs + evac into xtsb staging
                if q in pre_quads:
                    xtsbs = []
                    for cs in range(4):
                        xtsb = xtsb_pool.tile(
                            [128, 4, 4, 128], F16, tag="xtsb"
                        )
                        nc.sync.dma_start(
                            xtsb[:], xt_d[4 * pre_index[q] + cs]
                        )
                        xtsbs.append(xtsb)
                    xtsbs_q[q] = xtsbs
                    return
                xs = xs_all[q // 2]
                xtsbs = []
                for cs in range(4):
                    p = 4 * (q % 2) + cs
                    cset, j = p // 2, p % 2
                    xsrc = xs[cset]
                    if "E" in ablate:
                        xtsbs.append(xtsb_static)
                        if "T" not in ablate:
                            for kk in range(2):
                                xtA = xtps_pool.tile(
                                    [128, 4, 2, 128], F16, tag="xt"
                                )
                                for k in range(2):
                                    for c in range(4):
                                        nc.tensor.matmul(
                                            xtA[:, c, k, :],
                                            xsrc[
                                                :, 4 * j + 2 * kk + k,
                                                128 * c : 128 * (c + 1),
                                            ],
                                            identr[:],
                                            is_transpose=True,
                                        )
                        continue
                    xtsb = xtsb_pool.tile([128, 4, 4, 128], F16, tag="xtsb")
                    for kk in range(2):
                        if "T" in ablate:
                            evac(
                                xtsb[:, :, 2 * kk : 2 * kk + 2, :],
                                xtA_static[kk][:],
                            )
                            continue
                        xtA = xtps_pool.tile([128, 4, 2, 128], F16, tag="xt")
                        for k in range(2):
                            for c in range(4):
                                nc.tensor.matmul(
                                    xtA[:, c, k, :],
                                    xsrc[
                                        :, 4 * j + 2 * kk + k,
                                        128 * c : 128 * (c + 1),
                                    ],
                                    identr[:],
                                    is_transpose=True,
                                )
                        evac(xtsb[:, :, 2 * kk : 2 * kk + 2, :], xtA[:])
                    xtsbs.append(xtsb)
                xtsbs_q[q] = xtsbs

            def emit_S(q):
                # scores matmuls for quad q (needs xtsbs_q[q] evacuated)
                xtsbs = xtsbs_q.pop(q)
                if "S" in ablate:
                    scps_q[q] = scps_static
                    return
                scps = scps_pool.tile([128, 512], F32, tag="scps")
                sm = 8 if conf.get("s8") else 32
                for ci in range(4):
                    for cs in range(4):
                        nc.tensor.matmul(
                            scps[32 * cs : 32 * cs + sm, :],
                            A4[:, ci, 0:sm],
                            xtsbs[cs][:, ci, :, :],
                            start=(ci == 0),
                            stop=(ci == 3),
                            skip_group_check=True,
                            tile_position=(0, 32 * cs),
                        )
                scps_q[q] = scps

            def emit_X(q):
                # softmax (exp + normalize) for quad q
                scps = scps_q.pop(q)
                if "X" in ablate:
                    ptq_q[q] = ptq_static
                    return
                denq = den_pool.tile([128, 2], F32, tag="den")
                eq = eq_pool.tile([128, 2, NPG], F16, tag="eq")
                for g in range(2):
                    nc.scalar.activation(
                        eq[:, g, :],
                        scps[:, NPG * g : NPG * (g + 1)],
                        EXP,
                        accum_out=denq[:, g : g + 1],
                    )
                rdenq = den_pool.tile([128, 2], F32, tag="rden")
                nc.vector.reciprocal(rdenq[:], denq[:])
                ptq = pt_pool.tile([128, 2, NPG], F16, tag="ptq")
                for g in range(2):
                    nc.vector.tensor_scalar_mul(
                        ptq[:, g, :], eq[:, g, :], rdenq[:, g : g + 1]
                    )
                ptq_q[q] = ptq

            def emit_P(q, p16):
                # p back to natural layout + block-diag scatter, quad q
                ptq = ptq_q.pop(q)
                for cs in range(4):
                    p = 4 * (q % 2) + cs
                    cset, j = p // 2, p % 2
                    if "P" in ablate:
                        base = p16[cset][:]
                        dst = _AP(
                            base.tensor,
                            base.offset + 130 * j,
                            [list(base.ap[0])] + [[65, 2], [32, 2], [4, H]],
                        )
                        nc.vector.tensor_copy(dst, ppg_static[:])
                        continue
                    ppg = ppg_pool.tile([128, 2, 2, H], F16, tag="ppg")
                    for g in range(2):
                        for k in range(2):
                            nc.tensor.matmul(
                                ppg[:, g, k, :],
                                ptq[
                                    32 * cs : 32 * cs + 8, g,
                                    128 * k : 128 * (k + 1),
                                ],
                                idrep8[32 * cs : 32 * cs + 8, :],
                                is_transpose=True,
                                tile_position=(32 * cs, 0),
                            )
                    base = p16[cset][:]
                    dst = _AP(
                        base.tensor,
                        base.offset + 130 * j,
                        [list(base.ap[0])] + [[65, 2], [32, 2], [4, H]],
                    )
                    nc.vector.tensor_copy(dst, ppg[:])

            def emit_L(sg, p16):
                # pooling + pooled evac/transpose for supergroup sg
                xs = xs_all.pop(sg)
                if "L" in ablate:
                    s2sb = s2sb_pool.tile([128, D], F16, tag="s2sb")
                    nc.vector.tensor_copy(s2sb[:], s2ps_static[:])
                    stps = tail_ps.tile([128, 4, 128], F16, tag="tail")
                    for c in range(4):
                        nc.tensor.matmul(
                            stps[:, c, :],
                            s2sb[:, 128 * c : 128 * (c + 1)],
                            identr[:],
                            is_transpose=True,
                        )
                    nc.vector.tensor_copy(STall[:, :, sg, :], stps[:])
                    return
                s2ps = s2ps_pool.tile([128, D], F32, tag="s2")
                if conf.get("lorder", "k") == "strip":
                    for cp in range(NSETS):
                        for k in range(8):
                            nc.tensor.matmul(
                                s2ps[32 * cp : 32 * cp + 32, :],
                                p16[cp][:, k, :],
                                xs[cp][:, k, :],
                                start=(k == 0),
                                stop=(k == 7),
                                skip_group_check=True,
                                tile_position=(0, 32 * cp),
                            )
                else:
                    for k in range(8):
                        for cp in range(NSETS):
                            nc.tensor.matmul(
                                s2ps[32 * cp : 32 * cp + 32, :],
                                p16[cp][:, k, :],
                                xs[cp][:, k, :],
                                start=(k == 0),
                                stop=(k == 7),
                                skip_group_check=True,
                                tile_position=(0, 32 * cp),
                            )
                s2sb = s2sb_pool.tile([128, D], F16, tag="s2sb")
                nc.vector.tensor_copy(s2sb[:], s2ps[:])
                stps = tail_ps.tile([128, 4, 128], F16, tag="tail")
                for c in range(4):
                    nc.tensor.matmul(
                        stps[:, c, :],
                        s2sb[:, 128 * c : 128 * (c + 1)],
                        identr[:],
                        is_transpose=True,
                    )
                nc.vector.tensor_copy(STall[:, :, sg, :], stps[:])

            def p16_for(q):
                sg = q // 2
                if n_sg > 1:
                    return P16all[(sg % 2) * NSETS : (sg % 2) * NSETS + NSETS]
                return P16all

            if variant == "dma":
                for sg in range(n_sg):
                    emit_dma(sg)
                finz = tail_sb.tile([n_graphs, D], F32, tag="finsb")
                nc.vector.memset(finz[:], 0.0)
                nc.sync.dma_start(out_d[:], finz[:])
            else:
                skew = conf.get("skew", 2)
                emit_dma(0)
                if n_sg > 1:
                    emit_dma(1)
                for q in range(n_q + skew):
                    if q < n_q:
                        if q % 2 == 0 and q // 2 + 2 < n_sg:
                            emit_dma(q // 2 + 2)
                        emit_T(q)
                    if skew >= 1 and 0 <= q - 1 < n_q:
                        emit_S(q - 1)
                        emit_X(q - 1)
                    elif skew == 0 and q < n_q:
                        emit_S(q)
                        emit_X(q)
                    qp = q - skew
                    if 0 <= qp < n_q:
                        emit_P(qp, p16_for(qp))
                        if qp % 2 == 1:
                            emit_L(qp // 2, p16_for(qp))

                # tail: per-head projection of pooled sums
                pool4 = tail_ps.tile([DH, H, n_graphs], F32, tag="tail")
                for h in range(H):
                    for c in range(4):
                        rhs = STall[:, c, :, :].rearrange(
                            "p s (cp h gl) -> p s cp h gl", cp=4, h=H, gl=4
                        )[:, :, :, h, :]
                        nc.tensor.matmul(
                            pool4[:, h, :],
                            WvT4[:, c, h, :],
                            rhs,
                            start=(c == 0),
                            stop=(c == 3),
                        )
                pool4sb = tail_sb.tile([DH, H, n_graphs], F32R, tag="p4sb")
                nc.vector.tensor_copy(pool4sb[:], pool4[:])
                finps = tail_ps.tile([n_graphs, D], F32, tag="tail")
                for h in range(H):
                    nc.tensor.matmul(
                        finps[:],
                        pool4sb[:, h, :],
                        Wout8[:, h, :],
                        start=(h == 0),
                        stop=(h == H - 1),
                    )
                finsb = tail_sb.tile(
                    [n_graphs, D], F32, tag="finsb", name="finsb"
                )
                nc.vector.tensor_copy(finsb[:], finps[:])
                if "O" in ablate:
                    nc.sync.dma_start(out_d[:], fin_static[:])
                elif repeat > 1 and conf.get("deferout", 1):
                    # write the PREVIOUS iteration's result: its finsb is
                    # long ready, so this DMA never stalls the sync ring
                    # between this iteration's x loads and the next's.
                    if prev_finsb:
                        nc.sync.dma_start(out_d[:], prev_finsb[0][:])
                    prev_finsb.clear()
                    prev_finsb.append(finsb)
                else:
                    nc.sync.dma_start(out_d[:], finsb[:])

        if prev_finsb:
            nc.sync.dma_start(out_d[:], prev_finsb[0][:])

    nc.compile()
    _strip_debug(nc)
    return nc


def _strip_debug(nc):
    for fn in nc.m.functions:
        for alloc in fn.allocations:
            try:
                for ml in alloc.memorylocations or []:
                    if getattr(ml, "ant_debug", None) is not None:
                        ml.ant_debug = None
            except Exception:
                pass
        for b in fn.blocks:
            for inst in b.instructions:
                try:
                    if inst.debug is not None:
                        inst.debug = None
                    if inst.bass_addl_debug is not None:
                        inst.bass_addl_debug = None
                except Exception:
                    pass


def _host_prep(query, W_in, b_in, W_out, b_out):
    scale = 1.0 / np.sqrt(DH)
    q = ((query @ W_in[:D].T + b_in[:D]) * scale).reshape(H, DH)
    Wk = W_in[D : 2 * D]
    A = (Wk.reshape(H, DH, D) * q[:, :, None]).sum(1).T.astype(np.float32)
    A4 = np.zeros((128, 4, 32), np.float32)
    A4[:, :, :H] = A.reshape(4, 128, H).transpose(1, 0, 2)
    WvT = W_in[2 * D :].T.astype(np.float32)
    WvT4 = np.ascontiguousarray(WvT.reshape(4, 128, H, DH).transpose(1, 0, 2, 3))
    WoutT = W_out.T.astype(np.float32)
    Wout8 = np.ascontiguousarray(WoutT.reshape(H, DH, D).transpose(1, 0, 2))
    bias = (W_out @ b_in[2 * D :] + b_out).astype(np.float32)
    idrep8 = np.zeros((128, H), np.float32)
    for c in range(4):
        idrep8[32 * c : 32 * c + H, :] = np.eye(H)
    return A4, WvT4, Wout8, bias, idrep8


def _numpy_fallback(x, batch, num_graphs, query, W_in, b_in, W_out, b_out):
    nb = int(num_graphs)
    scale = 1.0 / np.sqrt(DH)
    q = ((query @ W_in[:D].T + b_in[:D]) * scale).reshape(H, DH)
    k = (x @ W_in[D : 2 * D].T + b_in[D : 2 * D]).reshape(-1, H, DH)
    v = (x @ W_in[2 * D :].T + b_in[2 * D :]).reshape(-1, H, DH)
    scores = np.einsum("nhd,hd->nh", k, q)
    smax = np.full((nb, H), -np.inf, np.float32)
    np.maximum.at(smax, batch, scores)
    e = np.exp(scores - smax[batch])
    denom = np.zeros((nb, H), np.float32)
    np.add.at(denom, batch, e)
    p = e / denom[batch]
    pooled = np.zeros((nb, H, DH), np.float32)
    np.add.at(pooled, batch, p[:, :, None] * v)
    return (pooled.reshape(nb, D) @ W_out.T + b_out).astype(np.float32)


def kernel(**inputs):
    x = np.ascontiguousarray(np.asarray(inputs["x"], dtype=np.float32))
    batch = np.asarray(inputs["batch"]).astype(np.int64)
    num_graphs = int(np.asarray(inputs["num_graphs"]))
    query = np.asarray(inputs["query"], dtype=np.float32)
    W_in = np.asarray(inputs["W_in"], dtype=np.float32)
    b_in = np.asarray(inputs["b_in"], dtype=np.float32)
    W_out = np.asarray(inputs["W_out"], dtype=np.float32)
    b_out = np.asarray(inputs["b_out"], dtype=np.float32)

    regular = (
        x.shape == (N, D)
        and num_graphs == B
        and batch.shape == (N,)
        and np.array_equal(batch, np.repeat(np.arange(B, dtype=np.int64), NPG))
    )
    if not regular:
        return _numpy_fallback(
            x, batch, num_graphs, query, W_in, b_in, W_out, b_out
        )

    from concourse.bass_utils import run_bass_kernel_spmd

    A4, WvT4, Wout8, bias, idrep8 = _host_prep(query, W_in, b_in, W_out, b_out)

    if "prog" not in _CACHE:
        _CACHE["prog"] = _build(GPC)
    nc = _CACHE["prog"]

    in_maps = _in_maps(x, A4, WvT4, Wout8, idrep8, CONF)
    res = run_bass_kernel_spmd(nc, in_maps, list(range(CORES)))
    out = np.concatenate([res.results[c]["out"] for c in range(CORES)], axis=0)
    return (out + bias[None, :]).astype(np.float32)

